# revision 20
# baseline (speedup 1.0000x reference)
"""Trainium2 Bass kernel for nn_DecoderLayerWithContext (8-core SPMD).

Decoder layer: pre-LN causal self-attention + pre-LN cross-attention + pre-LN FFN,
all with residual adds.  B=2, T=S=2048, E=1024, H=16, D=64.

Distribution: row-parallel (sequence-parallel) over the 8 cores.  Each core owns
512 rows (per batch, T-chunks i and 15-i of 128 rows — balanced causal load).
K/V are computed from each core's own rows and AllGather'd (bf16); attention,
projections, LNs and FFN are then fully local.  No partition_id needed: the
per-core program is identical, per-core data (row shards + causal mask) differs.

Everything on-chip runs in a TRANSPOSED activation layout ([E, rows], E-chunks of
128 on partitions) which makes every matmul natural and requires zero on-chip
transposes.  Matmul dtypes: float32r (tf32-class, 1 cyc/row) for weight matmuls,
bf16 for attention internals; fp32 residual stream / softmax / LN stats.
"""
import numpy as np
import ml_dtypes

import concourse.bass as bass
import concourse.bacc as bacc
import concourse.mybir as mybir
import concourse.tile as tile
from concourse import bass_utils

dt = mybir.dt
AF = mybir.ActivationFunctionType
OP = mybir.AluOpType

NCORES = 8
B, T, E, H, D = 2, 2048, 1024, 16, 64
EC = E // 128          # 8 E-chunks of 128
HP = H // 2            # 8 head-pairs (2 heads = 128 partitions of kT/qT)
CH = T // 128          # 16 sequence chunks of 128
R = 2 * B * 128        # 512 rows per core (B batches x 2 chunks x 128)
FE = 4 * E // 128      # 32 FFN hidden chunks
SCALE = 1.0 / np.sqrt(E)
EPS = 1e-5
BF = ml_dtypes.bfloat16


def _self_chunk_loc(c):
    """AG-buffer location of self-attn key chunk c: (rank, sel)."""
    return (c, 0) if c <= 7 else (15 - c, 1)


def _cross_chunk_loc(c):
    return (c // 2, c % 2)


def _build():
    nc = bacc.Bacc("TRN2", target_bir_lowering=False, debug=False,
                   num_devices=NCORES)

    xT = nc.dram_tensor("xT", [E, R], dt.float32, kind="ExternalInput").ap()
    ctxT = nc.dram_tensor("ctxT", [E, R], dt.float32r, kind="ExternalInput").ap()
    mbh = nc.dram_tensor("mbh", [128, CH * 256], dt.bfloat16, kind="ExternalInput").ap()
    wts = {}
    for name in ["Wq_s", "Wk_s", "Wv_s", "proj_s_w", "Wq_c", "Wk_c", "Wv_c", "proj_c_w"]:
        wts[name] = nc.dram_tensor(name, [E, E], dt.float32r, kind="ExternalInput").ap()
    w1 = nc.dram_tensor("w1", [E, 4 * E], dt.float32r, kind="ExternalInput").ap()
    w2 = nc.dram_tensor("w2", [4 * E, E], dt.float32r, kind="ExternalInput").ap()
    smalls = {}
    for name in ["ln1_g", "ln1_b", "ln2_g", "ln2_b", "ln3_g", "ln3_b",
                 "proj_s_b", "proj_c_b", "b2"]:
        smalls[name] = nc.dram_tensor(name + "_r", [128, EC], dt.float32,
                                      kind="ExternalInput").ap()
    b1r = nc.dram_tensor("b1_r", [128, FE], dt.float32, kind="ExternalInput").ap()
    ones_col_in = nc.dram_tensor("ones_col", [128, 1], dt.float32r,
                                 kind="ExternalInput").ap()
    ones_row_in = nc.dram_tensor("ones_row", [1, 128], dt.float32r,
                                 kind="ExternalInput").ap()
    selAB_in = nc.dram_tensor("selAB_in", [2, 128], dt.float32r,
                              kind="ExternalInput").ap()
    outT = nc.dram_tensor("outT", [E, R], dt.float32, kind="ExternalOutput").ap()

    consts = (ones_col_in, ones_row_in, selAB_in)
    with tile.TileContext(nc) as tc:
        _emit(nc, tc, xT, ctxT, mbh, wts, w1, w2, smalls, b1r, consts, outT)
    nc.compile()
    return nc


def _emit(nc, tc, xT, ctxT, mbh, wts, w1, w2, smalls, b1r, consts, outT):
    ones_col_in, ones_row_in, selAB_in = consts
    with tc.tile_pool(name="res", bufs=1) as res, \
         tc.tile_pool(name="dram", bufs=1, space="DRAM") as dram:
        # ---- resident tiles ----
        xt = res.tile([128, EC * R], dt.float32, name="xt")          # residual stream
        ht = res.tile([128, EC * R], dt.float32r, name="ht")         # LN output
        qt = res.tile([128, HP * R], dt.bfloat16, name="qt")         # queries (transposed)
        at = res.tile([128, HP * R], dt.float32r, name="at")         # attention out (transposed)
        mb = res.tile([128, CH * 256], dt.bfloat16, name="mb")       # causal mask (binary)
        gat = res.tile([128, 9 * EC], dt.float32, name="gat")        # gains/biases, col-packed
        b1t = res.tile([128, FE], dt.float32, name="b1t")
        # f32r constants come from the host (memset can't write float32r)
        ones_r = res.tile([128, 1], dt.float32r, name="ones_r")
        ones1 = res.tile([1, 128], dt.float32r, name="ones1")
        selA = res.tile([1, 128], dt.float32r, name="selA")
        selB = res.tile([1, 128], dt.float32r, name="selB")
        nc.sync.dma_start(ones_r[:], ones_col_in)
        nc.sync.dma_start(ones1[:], ones_row_in)
        nc.sync.dma_start(selA[:], selAB_in[0:1, :])
        nc.sync.dma_start(selB[:], selAB_in[1:2, :])
        ones_b = res.tile([128, 1], dt.bfloat16, name="ones_b")
        nc.gpsimd.memset(ones_b[:], 1.0)

        nc.sync.dma_start(mb[:], mbh)
        nc.sync.dma_start(b1t[:], b1r)
        small_names = ["ln1_g", "ln1_b", "ln2_g", "ln2_b", "ln3_g", "ln3_b",
                       "proj_s_b", "proj_c_b", "b2"]
        for idx, name in enumerate(small_names):
            nc.sync.dma_start(gat[:, idx * EC:(idx + 1) * EC], smalls[name])

        def gcol(name, e):
            idx = small_names.index(name)
            return gat[:, idx * EC + e: idx * EC + e + 1]

        for e in range(EC):
            nc.sync.dma_start(xt[:, e * R:(e + 1) * R], xT[e * 128:(e + 1) * 128, :])

        # ctx in f32r, transposed layout
        ctx = res.tile([128, EC * R], dt.float32r, name="ctx")
        for e in range(EC):
            nc.sync.dma_start(ctx[:, e * R:(e + 1) * R], ctxT[e * 128:(e + 1) * 128, :])

        # AG buffers
        kv_own_s = dram.tile([2 * E, R], dt.bfloat16, name="kv_own_s")
        kv_ag_s = dram.tile([NCORES * 2 * E, R], dt.bfloat16, addr_space="Shared",
                            name="kv_ag_s")
        kv_own_c = dram.tile([2 * E, R], dt.bfloat16, name="kv_own_c")
        kv_ag_c = dram.tile([NCORES * 2 * E, R], dt.bfloat16, addr_space="Shared",
                            name="kv_ag_c")

        # ---------- helpers ----------
        def load_weight(pool, w_ap, name):
            """[E, E] f32r DRAM -> [128, EC*E] sbuf (E-chunk e at cols e*E)."""
            wt = pool.tile([128, EC * E], dt.float32r, name=name, tag="wbig")
            nc.sync.dma_start(
                wt[:], w_ap.rearrange("(ec p) n -> p ec n", p=128))
            return wt

        def layer_norm(g_name, b_name):
            """ht = LN(xt) * g + b, computed in transposed layout."""
            with tc.tile_pool(name="lnp", bufs=1, space="PSUM") as lpp, \
                 tc.tile_pool(name="lns", bufs=1) as lsp:
                ps_sum = lpp.tile([1, R], dt.float32, name="ps_sum")
                ps_sq = lpp.tile([1, R], dt.float32, name="ps_sq")
                for e in range(EC):
                    xtr = lsp.tile([128, R], dt.float32r, name="xtr", tag="xtr", bufs=2)
                    sq = lsp.tile([128, R], dt.float32r, name="sq", tag="sq", bufs=2)
                    xc = xt[:, e * R:(e + 1) * R]
                    nc.scalar.activation(xtr[:], xc, AF.Copy)
                    nc.scalar.activation(sq[:], xc, AF.Square)
                    nc.tensor.matmul(ps_sum[:], ones_r[:], xtr[:],
                                     start=(e == 0), stop=(e == EC - 1))
                    nc.tensor.matmul(ps_sq[:], ones_r[:], sq[:],
                                     start=(e == 0), stop=(e == EC - 1))
                mu = lsp.tile([1, R], dt.float32, name="mu")
                ex2 = lsp.tile([1, R], dt.float32, name="ex2")
                var = lsp.tile([1, R], dt.float32, name="var")
                sd = lsp.tile([1, R], dt.float32, name="sd")
                rs = lsp.tile([1, R], dt.float32, name="rs")
                murs = lsp.tile([1, R], dt.float32, name="murs")
                rs_r = lsp.tile([1, R], dt.float32r, name="rs_r")
                murs_r = lsp.tile([1, R], dt.float32r, name="murs_r")
                musq = lsp.tile([1, R], dt.float32, name="musq")
                nc.vector.tensor_scalar_mul(mu[:], ps_sum[:], 1.0 / E)
                nc.vector.tensor_scalar_mul(ex2[:], ps_sq[:], 1.0 / E)
                nc.vector.tensor_mul(musq[:], mu[:], mu[:])
                # var + eps = (E[x^2] + eps) - mu^2
                nc.vector.scalar_tensor_tensor(var[:], ex2[:], EPS, musq[:],
                                               op0=OP.add, op1=OP.subtract)
                nc.scalar.activation(sd[:], var[:], AF.Sqrt)
                nc.vector.reciprocal(rs[:], sd[:])
                nc.vector.tensor_mul(murs[:], mu[:], rs[:])
                nc.scalar.activation(rs_r[:], rs[:], AF.Copy)
                nc.scalar.activation(murs_r[:], murs[:], AF.Copy)
                ps_rs = lpp.tile([128, R], dt.float32, name="ps_rs")
                ps_mu = lpp.tile([128, R], dt.float32, name="ps_mu")
                nc.tensor.matmul(ps_rs[:], ones1[:], rs_r[:], start=True, stop=True)
                nc.tensor.matmul(ps_mu[:], ones1[:], murs_r[:], start=True, stop=True)
                for e in range(EC):
                    t1 = lsp.tile([128, R], dt.float32, name="t1", tag="t1", bufs=2)
                    xc = xt[:, e * R:(e + 1) * R]
                    hc = ht[:, e * R:(e + 1) * R]
                    nc.vector.tensor_mul(t1[:], xc, ps_rs[:])
                    nc.vector.tensor_sub(t1[:], t1[:], ps_mu[:])
                    nc.vector.tensor_scalar(hc, t1[:], gcol(g_name, e),
                                            gcol(b_name, e), op0=OP.mult, op1=OP.add)

        def qkv_transposed(wt, rhs, out_bf, psp, csp):
            """out_bf [128, HP*R] bf16 = (rhs_E-chunks^T @ W) transposed: head-pair
            hp at cols hp*R."""
            for hp in range(HP):
                ps = psp.tile([128, R], dt.float32, name="qkvp", tag="qkvp", bufs=2)
                for e in range(EC):
                    nc.tensor.matmul(ps[:], wt[:, e * E + hp * 128: e * E + (hp + 1) * 128],
                                     rhs[:, e * R:(e + 1) * R],
                                     start=(e == 0), stop=(e == EC - 1))
                nc.scalar.activation(out_bf[:, hp * R:(hp + 1) * R], ps[:], AF.Copy)

        def v_normal(wt, lhs, out_bf, psp, csp):
            """out_bf [128, 4*E] bf16: V in normal layout, row-chunk r4 at cols r4*E."""
            for r4 in range(4):
                for nh in range(2):
                    ps = psp.tile([128, 512], dt.float32, name="vp", tag="qkvp", bufs=2)
                    for e in range(EC):
                        nc.tensor.matmul(
                            ps[:],
                            lhs[:, e * R + r4 * 128: e * R + (r4 + 1) * 128],
                            wt[:, e * E + nh * 512: e * E + (nh + 1) * 512],
                            start=(e == 0), stop=(e == EC - 1))
                    nc.scalar.activation(
                        out_bf[:, r4 * E + nh * 512: r4 * E + (nh + 1) * 512],
                        ps[:], AF.Copy)

        def store_kv(kt_bf, v_bf, kv_own):
            for hp in range(HP):
                nc.sync.dma_start(kv_own[hp * 128:(hp + 1) * 128, :],
                                  kt_bf[:, hp * R:(hp + 1) * R])
            for r4 in range(4):
                dst = kv_own[E + r4 * 256: E + (r4 + 1) * 256, :]
                nc.sync.dma_start(dst.rearrange("(p two) f -> p two f", two=2),
                                  v_bf[:, r4 * E:(r4 + 1) * E])

        def attention(kv_ag, chunk_loc, use_mask):
            """Consumes qt, writes at.  kv_ag is the gathered [NC*2E, R] buffer."""
            with tc.tile_pool(name="apsA", bufs=1, space="PSUM") as psA, \
                 tc.tile_pool(name="asb", bufs=1) as asb:
                for b in range(B):
                    for hp in range(HP):
                        kt_t = asb.tile([128, CH * 128], dt.bfloat16, name="kt_t",
                                        tag="kt_t", bufs=2)
                        vt_t = asb.tile([128, CH * 128], dt.bfloat16, name="vt_t",
                                        tag="vt_t", bufs=2)
                        for c in range(CH):
                            r, sel = chunk_loc(c)
                            col = (b * 2 + sel) * 128
                            nc.sync.dma_start(
                                kt_t[:, c * 128:(c + 1) * 128],
                                kv_ag[r * 2 * E + hp * 128: r * 2 * E + (hp + 1) * 128,
                                      col: col + 128])
                            vbase = r * 2 * E + E + (b * 2 + sel) * 256 + (hp // 4)
                            nc.sync.dma_start(
                                vt_t[:, c * 128:(c + 1) * 128],
                                kv_ag[vbase: vbase + 255: 2,
                                      (hp % 4) * 128: (hp % 4 + 1) * 128])
                        ot = psA.tile([128, 256], dt.float32, name="ot", tag="ot", bufs=2)
                        den = psA.tile([64, 256], dt.float32, name="den", tag="den", bufs=1)
                        qA = qt[0:64, hp * R + b * 256: hp * R + (b + 1) * 256]
                        qB = qt[64:128, hp * R + b * 256: hp * R + (b + 1) * 256]
                        for c in range(CH):
                            sA = psA.tile([128, 256], dt.float32, name="sA", tag="sA", bufs=2)
                            sB = psA.tile([128, 256], dt.float32, name="sB", tag="sB", bufs=2)
                            nc.tensor.matmul(sA[:], kt_t[0:64, c * 128:(c + 1) * 128], qA,
                                             start=True, stop=True, tile_position=(0, 0))
                            nc.tensor.matmul(sB[:], kt_t[64:128, c * 128:(c + 1) * 128], qB,
                                             start=True, stop=True, tile_position=(64, 0))
                            pA = asb.tile([128, 256], dt.bfloat16, name="pA", tag="pA", bufs=3)
                            pB = asb.tile([128, 256], dt.bfloat16, name="pB", tag="pB", bufs=3)
                            nc.scalar.activation(pA[:], sA[:], AF.Exp, scale=SCALE)
                            nc.scalar.activation(pB[:], sB[:], AF.Exp, scale=SCALE)
                            if use_mask:
                                mslice = mb[:, c * 256:(c + 1) * 256]
                                nc.vector.tensor_mul(pA[:], pA[:], mslice)
                                nc.vector.tensor_mul(pB[:], pB[:], mslice)
                            nc.tensor.matmul(ot[0:64, :], vt_t[:, c * 128: c * 128 + 64],
                                             pA[:], start=(c == 0), stop=(c == CH - 1),
                                             tile_position=(0, 0))
                            nc.tensor.matmul(ot[64:128, :], vt_t[:, c * 128 + 64: (c + 1) * 128],
                                             pB[:], start=(c == 0), stop=(c == CH - 1),
                                             tile_position=(0, 64))
                            nc.tensor.matmul(den[0:1, :], ones_b[:], pA[:],
                                             start=(c == 0), stop=(c == CH - 1),
                                             tile_position=(0, 0))
                            nc.tensor.matmul(den[32:33, :], ones_b[:], pB[:],
                                             start=(c == 0), stop=(c == CH - 1),
                                             tile_position=(0, 32))
                        rA2 = asb.tile([1, 256], dt.float32r, name="rA2", tag="rA2", bufs=2)
                        rB2 = asb.tile([1, 256], dt.float32r, name="rB2", tag="rB2", bufs=2)
                        with nc.allow_low_precision(reason="softmax denom recip"):
                            nc.vector.reciprocal(rA2[:], den[0:1, :])
                            nc.vector.reciprocal(rB2[:], den[32:33, :])
                        rb = psA.tile([128, 256], dt.float32, name="rb", tag="rb", bufs=1)
                        nc.tensor.matmul(rb[:], selA[:], rA2[:], start=True, stop=False)
                        nc.tensor.matmul(rb[:], selB[:], rB2[:], start=False, stop=True)
                        rbs = asb.tile([128, 256], dt.float32, name="rbs", tag="rbs", bufs=2)
                        nc.scalar.activation(rbs[:], rb[:], AF.Copy)
                        nc.vector.tensor_mul(
                            at[:, hp * R + b * 256: hp * R + (b + 1) * 256],
                            ot[:], rbs[:])

        def proj_residual(wt, bias_name, psp):
            for pe in range(EC):
                pp = psp.tile([128, R], dt.float32, name="pp", tag="qkvp", bufs=2)
                for hp in range(HP):
                    nc.tensor.matmul(pp[:], wt[:, hp * E + pe * 128: hp * E + (pe + 1) * 128],
                                     at[:, hp * R:(hp + 1) * R],
                                     start=(hp == 0), stop=(hp == HP - 1))
                xc = xt[:, pe * R:(pe + 1) * R]
                nc.vector.scalar_tensor_tensor(xc, pp[:], gcol(bias_name, pe), xc,
                                               op0=OP.add, op1=OP.add)

        # ================= program =================
        # LN1 + self KV -> AG(self);  cross KV -> AG(cross);  qT self.
        layer_norm("ln1_g", "ln1_b")
        with tc.tile_pool(name="qkvps", bufs=1, space="PSUM") as psp, \
             tc.tile_pool(name="wpool", bufs=2) as wp, \
             tc.tile_pool(name="kvsb", bufs=1) as kvsb:
            kbuf = kvsb.tile([128, HP * R], dt.bfloat16, name="kbuf", tag="kv", bufs=2)
            vbuf = kvsb.tile([128, 4 * E], dt.bfloat16, name="vbuf", tag="kv", bufs=2)
            wk = load_weight(wp, wts["Wk_s"], "wk_s")
            qkv_transposed(wk, ht, kbuf, psp, kvsb)
            store_kv_k = kbuf
            wv = load_weight(wp, wts["Wv_s"], "wv_s")
            v_normal(wv, ht, vbuf, psp, kvsb)
            store_kv(store_kv_k, vbuf, kv_own_s)
            nc.gpsimd.collective_compute(
                "AllGather", OP.bypass,
                replica_groups=[list(range(NCORES))],
                ins=[kv_own_s[:]], outs=[kv_ag_s[:]])

            wq = load_weight(wp, wts["Wq_s"], "wq_s")
            qkv_transposed(wq, ht, qt, psp, kvsb)

            # cross K/V from context (independent of x) — overlaps self-attn wait
            kbuf2 = kvsb.tile([128, HP * R], dt.bfloat16, name="kbuf2", tag="kv", bufs=2)
            vbuf2 = kvsb.tile([128, 4 * E], dt.bfloat16, name="vbuf2", tag="kv", bufs=2)
            wkc = load_weight(wp, wts["Wk_c"], "wk_c")
            qkv_transposed(wkc, ctx, kbuf2, psp, kvsb)
            wvc = load_weight(wp, wts["Wv_c"], "wv_c")
            v_normal(wvc, ctx, vbuf2, psp, kvsb)
            store_kv(kbuf2, vbuf2, kv_own_c)
            nc.gpsimd.collective_compute(
                "AllGather", OP.bypass,
                replica_groups=[list(range(NCORES))],
                ins=[kv_own_c[:]], outs=[kv_ag_c[:]])

        # self-attention
        attention(kv_ag_s, _self_chunk_loc, use_mask=True)
        with tc.tile_pool(name="prjps", bufs=1, space="PSUM") as psp, \
             tc.tile_pool(name="wpool2", bufs=2) as wp:
            wps = load_weight(wp, wts["proj_s_w"], "wproj_s")
            proj_residual(wps, "proj_s_b", psp)

        # cross-attention
        layer_norm("ln2_g", "ln2_b")
        with tc.tile_pool(name="qkvps2", bufs=1, space="PSUM") as psp, \
             tc.tile_pool(name="wpool3", bufs=2) as wp:
            wqc = load_weight(wp, wts["Wq_c"], "wq_c")
            qkv_transposed(wqc, ht, qt, psp, None)
        attention(kv_ag_c, _cross_chunk_loc, use_mask=False)
        with tc.tile_pool(name="prjps2", bufs=1, space="PSUM") as psp, \
             tc.tile_pool(name="wpool4", bufs=2) as wp:
            wpc = load_weight(wp, wts["proj_c_w"], "wproj_c")
            proj_residual(wpc, "proj_c_b", psp)

        # FFN
        layer_norm("ln3_g", "ln3_b")
        with tc.tile_pool(name="ffnps", bufs=1, space="PSUM") as psp, \
             tc.tile_pool(name="ffnsb", bufs=1) as fsb:
            h4g = fsb.tile([128, FE * R], dt.float32r, name="h4g")
            for fe in range(FE):
                w1t = fsb.tile([128, E], dt.float32r, name="w1t", tag="w1t", bufs=2)
                nc.sync.dma_start(
                    w1t[:],
                    w1[:, fe * 128:(fe + 1) * 128].rearrange("(ec p) c -> p ec c", p=128))
                ph = psp.tile([128, R], dt.float32, name="ph", tag="qkvp", bufs=2)
                for e in range(EC):
                    nc.tensor.matmul(ph[:], w1t[:, e * 128:(e + 1) * 128],
                                     ht[:, e * R:(e + 1) * R],
                                     start=(e == 0), stop=(e == EC - 1))
                nc.scalar.activation(h4g[:, fe * R:(fe + 1) * R], ph[:], AF.Gelu,
                                     bias=b1t[:, fe:fe + 1])
            for pe in range(EC):
                w2t = fsb.tile([128, FE * 128], dt.float32r, name="w2t", tag="w2t", bufs=2)
                nc.sync.dma_start(
                    w2t[:],
                    w2[:, pe * 128:(pe + 1) * 128].rearrange("(fc p) c -> p fc c", p=128))
                po = psp.tile([128, R], dt.float32, name="po", tag="qkvp", bufs=2)
                for fe in range(FE):
                    nc.tensor.matmul(po[:], w2t[:, fe * 128:(fe + 1) * 128],
                                     h4g[:, fe * R:(fe + 1) * R],
                                     start=(fe == 0), stop=(fe == FE - 1))
                xc = xt[:, pe * R:(pe + 1) * R]
                nc.vector.scalar_tensor_tensor(xc, po[:], gcol("b2", pe), xc,
                                               op0=OP.add, op1=OP.add)
                nc.sync.dma_start(outT[pe * 128:(pe + 1) * 128, :], xc)


# ---------------- host side ----------------

_CACHED_NC = None


def _get_nc():
    global _CACHED_NC
    if _CACHED_NC is None:
        _CACHED_NC = _build()
    return _CACHED_NC


def _row_slices(core):
    """Per-batch (lo, hi) T-chunk row ranges owned by `core`."""
    lo, hi = core, 15 - core
    return (slice(lo * 128, (lo + 1) * 128), slice(hi * 128, (hi + 1) * 128))


def _shard_inputs(inputs):
    f32 = np.float32
    x = np.asarray(inputs["x"], f32)
    context = np.asarray(inputs["context"], f32)
    reshape_small = lambda v: np.ascontiguousarray(
        np.asarray(v, f32).reshape(-1, 128).T)
    small_r = {
        "ln1_g_r": reshape_small(inputs["ln1_g"]), "ln1_b_r": reshape_small(inputs["ln1_b"]),
        "ln2_g_r": reshape_small(inputs["ln2_g"]), "ln2_b_r": reshape_small(inputs["ln2_b"]),
        "ln3_g_r": reshape_small(inputs["ln3_g"]), "ln3_b_r": reshape_small(inputs["ln3_b"]),
        "proj_s_b_r": reshape_small(inputs["proj_s_b"]),
        "proj_c_b_r": reshape_small(inputs["proj_c_b"]),
        "b2_r": reshape_small(inputs["b2"]),
        "b1_r": reshape_small(inputs["b1"]),
    }
    selAB = np.zeros((2, 128), f32)
    selAB[0, 0:64] = 1.0
    selAB[1, 64:128] = 1.0
    shared = {
        "ones_col": np.ones((128, 1), f32),
        "ones_row": np.ones((1, 128), f32),
        "selAB_in": selAB,
        "Wq_s": np.asarray(inputs["Wq_s"], f32), "Wk_s": np.asarray(inputs["Wk_s"], f32),
        "Wv_s": np.asarray(inputs["Wv_s"], f32), "proj_s_w": np.asarray(inputs["proj_s_w"], f32),
        "Wq_c": np.asarray(inputs["Wq_c"], f32), "Wk_c": np.asarray(inputs["Wk_c"], f32),
        "Wv_c": np.asarray(inputs["Wv_c"], f32), "proj_c_w": np.asarray(inputs["proj_c_w"], f32),
        "w1": np.asarray(inputs["w1"], f32), "w2": np.asarray(inputs["w2"], f32),
        **small_r,
    }
    in_maps = []
    kk = np.arange(T)[:, None]
    for core in range(NCORES):
        lo_sl, hi_sl = _row_slices(core)
        xrows = np.concatenate(
            [x[0, lo_sl], x[0, hi_sl], x[1, lo_sl], x[1, hi_sl]], axis=0)
        ctxrows = np.concatenate(
            [context[0, core * 256:(core + 1) * 256],
             context[1, core * 256:(core + 1) * 256]], axis=0)
        qpos = np.concatenate([np.arange(core * 128, (core + 1) * 128),
                               np.arange((15 - core) * 128, (16 - core) * 128)])
        maskbin = (kk <= qpos[None, :]).astype(f32)  # [T, 256]
        # mbh[p, c*256+q] = maskbin[c*128+p, q]
        mbh = np.ascontiguousarray(
            maskbin.reshape(CH, 128, 256).transpose(1, 0, 2).reshape(128, CH * 256)
        ).astype(BF)
        in_maps.append({
            "xT": np.ascontiguousarray(xrows.T),
            "ctxT": np.ascontiguousarray(ctxrows.T),
            "mbh": mbh,
            **shared,
        })
    return in_maps


def _unshard_output(results):
    out = np.empty((B, T, E), np.float32)
    for core in range(NCORES):
        rows = results[core]["outT"].T  # [512, E]
        lo_sl, hi_sl = _row_slices(core)
        out[0, lo_sl] = rows[0:128]
        out[0, hi_sl] = rows[128:256]
        out[1, lo_sl] = rows[256:384]
        out[1, hi_sl] = rows[384:512]
    return out


def kernel(**inputs):
    nc = _get_nc()
    in_maps = _shard_inputs(inputs)
    res = bass_utils.run_bass_kernel_spmd(nc, in_maps, core_ids=list(range(NCORES)))
    return _unshard_output(res.results)


if __name__ == "__main__":
    # smoke test with random inputs
    rng = np.random.default_rng(0)
    dummy = {
        "x": rng.standard_normal((B, T, E), dtype=np.float32),
        "context": rng.standard_normal((B, T, E), dtype=np.float32),
    }
    for n in ["ln1", "ln2", "ln3"]:
        dummy[n + "_g"] = np.ones(E, np.float32)
        dummy[n + "_b"] = np.zeros(E, np.float32)
    for n in ["Wq_s", "Wk_s", "Wv_s", "proj_s_w", "Wq_c", "Wk_c", "Wv_c", "proj_c_w"]:
        dummy[n] = (rng.standard_normal((E, E), dtype=np.float32) * 0.02)
    dummy["proj_s_b"] = np.zeros(E, np.float32)
    dummy["proj_c_b"] = np.zeros(E, np.float32)
    dummy["w1"] = rng.standard_normal((E, 4 * E), dtype=np.float32) * 0.02
    dummy["b1"] = np.zeros(4 * E, np.float32)
    dummy["w2"] = rng.standard_normal((4 * E, E), dtype=np.float32) * 0.02
    dummy["b2"] = np.zeros(E, np.float32)
    out = kernel(**dummy)
    print("out", out.shape, out.dtype, np.abs(out).mean())


# revision 23
# speedup vs baseline: 1.1834x; 1.1834x over previous
"""Trainium2 Bass kernel for nn_DecoderLayerWithContext (8-core SPMD).

Decoder layer: pre-LN causal self-attention + pre-LN cross-attention + pre-LN FFN,
all with residual adds.  B=2, T=S=2048, E=1024, H=16, D=64.

Distribution: row-parallel (sequence-parallel) over the 8 cores.  Each core owns
512 rows (per batch, T-chunks i and 15-i of 128 rows — balanced causal load).
K/V are computed from each core's own rows and AllGather'd (bf16); attention,
projections, LNs and FFN are then fully local.  No partition_id needed: the
per-core program is identical, per-core data (row shards + causal mask) differs.

Everything on-chip runs in a TRANSPOSED activation layout ([E, rows], E-chunks of
128 on partitions) which makes every matmul natural and requires zero on-chip
transposes.  Matmul dtypes: float32r (tf32-class, 1 cyc/row) for weight matmuls,
bf16 for attention internals; fp32 residual stream / softmax / LN stats.
"""
import numpy as np
import ml_dtypes

import concourse.bass as bass
import concourse.bacc as bacc
import concourse.mybir as mybir
import concourse.tile as tile
from concourse import bass_utils

dt = mybir.dt
AF = mybir.ActivationFunctionType
OP = mybir.AluOpType

NCORES = 8
B, T, E, H, D = 2, 2048, 1024, 16, 64
EC = E // 128          # 8 E-chunks of 128
HP = H // 2            # 8 head-pairs (2 heads = 128 partitions of kT/qT)
CH = T // 128          # 16 sequence chunks of 128
R = 2 * B * 128        # 512 rows per core (B batches x 2 chunks x 128)
FE = 4 * E // 128      # 32 FFN hidden chunks
SCALE = 1.0 / np.sqrt(E)
EPS = 1e-5
BF = ml_dtypes.bfloat16


def _self_chunk_loc(c):
    """AG-buffer location of self-attn key chunk c: (rank, sel)."""
    return (c, 0) if c <= 7 else (15 - c, 1)


def _cross_chunk_loc(c):
    return (c // 2, c % 2)


def _build():
    nc = bacc.Bacc("TRN2", target_bir_lowering=False, debug=False,
                   num_devices=NCORES)

    xT = nc.dram_tensor("xT", [E, R], dt.float32, kind="ExternalInput").ap()
    ctxT = nc.dram_tensor("ctxT", [E, R], dt.float32r, kind="ExternalInput").ap()
    mbh = nc.dram_tensor("mbh", [128, CH * 256], dt.bfloat16, kind="ExternalInput").ap()
    wts = {}
    for name in ["Wq_s", "Wk_s", "Wv_s", "proj_s_w", "Wq_c", "Wk_c", "Wv_c", "proj_c_w"]:
        wts[name] = nc.dram_tensor(name, [E, E], dt.float32r, kind="ExternalInput").ap()
    w1 = nc.dram_tensor("w1", [E, 4 * E], dt.float32r, kind="ExternalInput").ap()
    w2 = nc.dram_tensor("w2", [4 * E, E], dt.float32r, kind="ExternalInput").ap()
    smalls = {}
    for name in ["ln1_g", "ln1_b", "ln2_g", "ln2_b", "ln3_g", "ln3_b",
                 "proj_s_b", "proj_c_b", "b2"]:
        smalls[name] = nc.dram_tensor(name + "_r", [128, EC], dt.float32,
                                      kind="ExternalInput").ap()
    b1r = nc.dram_tensor("b1_r", [128, FE], dt.float32, kind="ExternalInput").ap()
    ones_col_in = nc.dram_tensor("ones_col", [128, 1], dt.float32r,
                                 kind="ExternalInput").ap()
    ones_row_in = nc.dram_tensor("ones_row", [1, 128], dt.float32r,
                                 kind="ExternalInput").ap()
    selAB_in = nc.dram_tensor("selAB_in", [2, 128], dt.float32r,
                              kind="ExternalInput").ap()
    outT = nc.dram_tensor("outT", [E, R], dt.float32, kind="ExternalOutput").ap()

    consts = (ones_col_in, ones_row_in, selAB_in)
    with tile.TileContext(nc) as tc:
        _emit(nc, tc, xT, ctxT, mbh, wts, w1, w2, smalls, b1r, consts, outT)
    nc.compile()
    return nc


def _emit(nc, tc, xT, ctxT, mbh, wts, w1, w2, smalls, b1r, consts, outT):
    ones_col_in, ones_row_in, selAB_in = consts
    with tc.tile_pool(name="res", bufs=1) as res, \
         tc.tile_pool(name="dram", bufs=1, space="DRAM") as dram:
        # ---- resident tiles ----
        xt = res.tile([128, EC * R], dt.float32, name="xt")          # residual stream
        ht = res.tile([128, EC * R], dt.float32r, name="ht")         # LN output
        qt = res.tile([128, HP * R], dt.bfloat16, name="qt")         # queries (transposed)
        at = res.tile([128, HP * R], dt.float32r, name="at")         # attention out (transposed)
        mb = res.tile([128, CH * 256], dt.bfloat16, name="mb")       # causal mask (binary)
        gat = res.tile([128, 9 * EC], dt.float32, name="gat")        # gains/biases, col-packed
        b1t = res.tile([128, FE], dt.float32, name="b1t")
        # f32r constants come from the host (memset can't write float32r)
        ones_r = res.tile([128, 1], dt.float32r, name="ones_r")
        ones1 = res.tile([1, 128], dt.float32r, name="ones1")
        selA = res.tile([1, 128], dt.float32r, name="selA")
        selB = res.tile([1, 128], dt.float32r, name="selB")
        nc.sync.dma_start(ones_r[:], ones_col_in)
        nc.sync.dma_start(ones1[:], ones_row_in)
        nc.sync.dma_start(selA[:], selAB_in[0:1, :])
        nc.sync.dma_start(selB[:], selAB_in[1:2, :])

        nc.sync.dma_start(mb[:], mbh)
        nc.sync.dma_start(b1t[:], b1r)
        small_names = ["ln1_g", "ln1_b", "ln2_g", "ln2_b", "ln3_g", "ln3_b",
                       "proj_s_b", "proj_c_b", "b2"]
        for idx, name in enumerate(small_names):
            nc.sync.dma_start(gat[:, idx * EC:(idx + 1) * EC], smalls[name])

        def gcol(name, e):
            idx = small_names.index(name)
            return gat[:, idx * EC + e: idx * EC + e + 1]

        for e in range(EC):
            nc.sync.dma_start(xt[:, e * R:(e + 1) * R], xT[e * 128:(e + 1) * 128, :])

        # ctx in f32r, transposed layout
        ctx = res.tile([128, EC * R], dt.float32r, name="ctx")
        for e in range(EC):
            nc.sync.dma_start(ctx[:, e * R:(e + 1) * R], ctxT[e * 128:(e + 1) * 128, :])

        # AG buffers
        kv_own_s = dram.tile([2 * E, R], dt.bfloat16, name="kv_own_s")
        kv_ag_s = dram.tile([NCORES * 2 * E, R], dt.bfloat16, addr_space="Shared",
                            name="kv_ag_s")
        kv_own_c = dram.tile([2 * E, R], dt.bfloat16, name="kv_own_c")
        kv_ag_c = dram.tile([NCORES * 2 * E, R], dt.bfloat16, addr_space="Shared",
                            name="kv_ag_c")

        # ---------- helpers ----------
        def load_weight(pool, w_ap, name):
            """[E, E] f32r DRAM -> [128, EC*E] sbuf (E-chunk e at cols e*E)."""
            wt = pool.tile([128, EC * E], dt.float32r, name=name, tag="wbig")
            nc.sync.dma_start(
                wt[:], w_ap.rearrange("(ec p) n -> p ec n", p=128))
            return wt

        def layer_norm(g_name, b_name):
            """ht = LN(xt) * g + b, computed in transposed layout."""
            with tc.tile_pool(name="lnp", bufs=1, space="PSUM") as lpp, \
                 tc.tile_pool(name="lns", bufs=1) as lsp:
                ps_sum = lpp.tile([1, R], dt.float32, name="ps_sum")
                ps_sq = lpp.tile([1, R], dt.float32, name="ps_sq")
                for e in range(EC):
                    xtr = lsp.tile([128, R], dt.float32r, name="xtr", tag="xtr", bufs=2)
                    sq = lsp.tile([128, R], dt.float32r, name="sq", tag="sq", bufs=2)
                    xc = xt[:, e * R:(e + 1) * R]
                    nc.scalar.activation(xtr[:], xc, AF.Copy)
                    nc.scalar.activation(sq[:], xc, AF.Square)
                    nc.tensor.matmul(ps_sum[:], ones_r[:], xtr[:],
                                     start=(e == 0), stop=(e == EC - 1))
                    nc.tensor.matmul(ps_sq[:], ones_r[:], sq[:],
                                     start=(e == 0), stop=(e == EC - 1))
                mu = lsp.tile([1, R], dt.float32, name="mu")
                ex2 = lsp.tile([1, R], dt.float32, name="ex2")
                var = lsp.tile([1, R], dt.float32, name="var")
                sd = lsp.tile([1, R], dt.float32, name="sd")
                rs = lsp.tile([1, R], dt.float32, name="rs")
                murs = lsp.tile([1, R], dt.float32, name="murs")
                rs_r = lsp.tile([1, R], dt.float32r, name="rs_r")
                murs_r = lsp.tile([1, R], dt.float32r, name="murs_r")
                musq = lsp.tile([1, R], dt.float32, name="musq")
                nc.vector.tensor_scalar_mul(mu[:], ps_sum[:], 1.0 / E)
                nc.vector.tensor_scalar_mul(ex2[:], ps_sq[:], 1.0 / E)
                nc.vector.tensor_mul(musq[:], mu[:], mu[:])
                # var + eps = (E[x^2] + eps) - mu^2
                nc.vector.scalar_tensor_tensor(var[:], ex2[:], EPS, musq[:],
                                               op0=OP.add, op1=OP.subtract)
                nc.scalar.activation(sd[:], var[:], AF.Sqrt)
                nc.vector.reciprocal(rs[:], sd[:])
                nc.vector.tensor_mul(murs[:], mu[:], rs[:])
                nc.scalar.activation(rs_r[:], rs[:], AF.Copy)
                nc.scalar.activation(murs_r[:], murs[:], AF.Copy)
                ps_rs = lpp.tile([128, R], dt.float32, name="ps_rs")
                ps_mu = lpp.tile([128, R], dt.float32, name="ps_mu")
                nc.tensor.matmul(ps_rs[:], ones1[:], rs_r[:], start=True, stop=True)
                nc.tensor.matmul(ps_mu[:], ones1[:], murs_r[:], start=True, stop=True)
                for e in range(EC):
                    t1 = lsp.tile([128, R], dt.float32, name="t1", tag="t1", bufs=2)
                    xc = xt[:, e * R:(e + 1) * R]
                    hc = ht[:, e * R:(e + 1) * R]
                    nc.vector.tensor_mul(t1[:], xc, ps_rs[:])
                    nc.vector.tensor_sub(t1[:], t1[:], ps_mu[:])
                    nc.vector.tensor_scalar(hc, t1[:], gcol(g_name, e),
                                            gcol(b_name, e), op0=OP.mult, op1=OP.add)

        def qkv_transposed(wt, rhs, out_bf, psp, csp):
            """out_bf [128, HP*R] bf16 = (rhs_E-chunks^T @ W) transposed: head-pair
            hp at cols hp*R."""
            for hp in range(HP):
                ps = psp.tile([128, R], dt.float32, name="qkvp", tag="qkvp", bufs=2)
                for e in range(EC):
                    nc.tensor.matmul(ps[:], wt[:, e * E + hp * 128: e * E + (hp + 1) * 128],
                                     rhs[:, e * R:(e + 1) * R],
                                     start=(e == 0), stop=(e == EC - 1))
                nc.scalar.activation(out_bf[:, hp * R:(hp + 1) * R], ps[:], AF.Copy)

        def v_normal(wt, lhs, out_bf, psp, csp):
            """out_bf [128, 4*E] bf16: V in normal layout, row-chunk r4 at cols r4*E."""
            for r4 in range(4):
                for nh in range(2):
                    ps = psp.tile([128, 512], dt.float32, name="vp", tag="qkvp", bufs=2)
                    for e in range(EC):
                        nc.tensor.matmul(
                            ps[:],
                            lhs[:, e * R + r4 * 128: e * R + (r4 + 1) * 128],
                            wt[:, e * E + nh * 512: e * E + (nh + 1) * 512],
                            start=(e == 0), stop=(e == EC - 1))
                    nc.scalar.activation(
                        out_bf[:, r4 * E + nh * 512: r4 * E + (nh + 1) * 512],
                        ps[:], AF.Copy)

        def store_kv(kt_bf, v_bf, kv_own):
            for hp in range(HP):
                nc.sync.dma_start(kv_own[hp * 128:(hp + 1) * 128, :],
                                  kt_bf[:, hp * R:(hp + 1) * R])
            for r4 in range(4):
                dst = kv_own[E + r4 * 256: E + (r4 + 1) * 256, :]
                nc.sync.dma_start(dst.rearrange("(p two) f -> p two f", two=2),
                                  v_bf[:, r4 * E:(r4 + 1) * E])

        def attention(kv_ag, use_mask):
            """Consumes qt, writes at.  kv_ag is the gathered [NC*2E, R] buffer.

            Key/value storage blocks j are rank-major: j<8 -> rank j sel 0,
            j>=8 -> rank j-8 sel 1.  The host mask uses the same permutation,
            so softmax (order-invariant over keys) is unaffected.
            V tiles carry an appended ones-column per head (lhsT [128, 65]) so
            the av matmul also produces the softmax denominator in row 64.
            """
            with tc.tile_pool(name="apsA", bufs=1, space="PSUM") as psA, \
                 tc.tile_pool(name="asb", bufs=1) as asb:
                rr = kv_ag[:].rearrange("(rk row) col -> row rk col", rk=NCORES)
                for b in range(B):
                    for hp in range(HP):
                        kt_t = asb.tile([128, CH * 128], dt.bfloat16, name="kt_t",
                                        tag="kt_t", bufs=2)
                        vt_t = asb.tile([128, CH * 130], dt.bfloat16, name="vt_t",
                                        tag="vt_t", bufs=2)
                        for sel in range(2):
                            col = (b * 2 + sel) * 128
                            nc.sync.dma_start(
                                kt_t[:, sel * 1024:(sel + 1) * 1024]
                                .rearrange("p (rk cc) -> p rk cc", rk=8),
                                rr[hp * 128:(hp + 1) * 128, :, col:col + 128])
                            vbase = E + (b * 2 + sel) * 256 + hp // 4
                            vsrc = rr[vbase:vbase + 255:2, :, :]
                            vdst = (vt_t[:, sel * 8 * 130:(sel + 1) * 8 * 130]
                                    .rearrange("p (rk z) -> p rk z", z=130))
                            for h in range(2):
                                nc.sync.dma_start(
                                    vdst[:, :, h * 65: h * 65 + 64],
                                    vsrc[:, :, (hp % 4) * 128 + h * 64:
                                         (hp % 4) * 128 + (h + 1) * 64])
                        nc.gpsimd.memset(vt_t[:, 64: CH * 130: 130], 1.0)
                        nc.gpsimd.memset(vt_t[:, 129: CH * 130: 130], 1.0)

                        otA = psA.tile([65, 256], dt.float32, name="otA", tag="otA", bufs=2)
                        otB = psA.tile([65, 256], dt.float32, name="otB", tag="otB", bufs=2)
                        qA = qt[0:64, hp * R + b * 256: hp * R + (b + 1) * 256]
                        qB = qt[64:128, hp * R + b * 256: hp * R + (b + 1) * 256]
                        for cp in range(CH // 2):
                            s2A = psA.tile([128, 512], dt.float32, name="s2A",
                                           tag="sAB", bufs=3)
                            s2B = psA.tile([128, 512], dt.float32, name="s2B",
                                           tag="sAB", bufs=3)
                            for ci in range(2):
                                blk = 2 * cp + ci
                                nc.tensor.matmul(
                                    s2A[:, ci * 256:(ci + 1) * 256],
                                    kt_t[0:64, blk * 128:(blk + 1) * 128], qA,
                                    start=True, stop=True, tile_position=(0, 0))
                                nc.tensor.matmul(
                                    s2B[:, ci * 256:(ci + 1) * 256],
                                    kt_t[64:128, blk * 128:(blk + 1) * 128], qB,
                                    start=True, stop=True, tile_position=(64, 0))
                            pA2 = asb.tile([128, 512], dt.bfloat16, name="pA2",
                                           tag="pA2", bufs=3)
                            pB2 = asb.tile([128, 512], dt.bfloat16, name="pB2",
                                           tag="pB2", bufs=3)
                            nc.scalar.activation(pA2[:], s2A[:], AF.Exp, scale=SCALE)
                            nc.scalar.activation(pB2[:], s2B[:], AF.Exp, scale=SCALE)
                            if use_mask:
                                ms = mb[:, cp * 512:(cp + 1) * 512]
                                nc.vector.tensor_mul(pA2[:], pA2[:], ms)
                                nc.vector.tensor_mul(pB2[:], pB2[:], ms)
                            for ci in range(2):
                                blk = 2 * cp + ci
                                st, sp = (blk == 0), (blk == CH - 1)
                                nc.tensor.matmul(
                                    otA[:], vt_t[:, blk * 130: blk * 130 + 65],
                                    pA2[:, ci * 256:(ci + 1) * 256], start=st, stop=sp)
                                nc.tensor.matmul(
                                    otB[:], vt_t[:, blk * 130 + 65:(blk + 1) * 130],
                                    pB2[:, ci * 256:(ci + 1) * 256], start=st, stop=sp)
                        # normalize: rows 64 of otA/otB hold the denominators
        # reciprocal_approx_fast reads garbage from PSUM at base partition
                        # 64 (probe4) — stage the denominator rows through SBUF.
                        denA = asb.tile([1, 256], dt.float32, name="denA",
                                        tag="denA", bufs=2)
                        denB = asb.tile([1, 256], dt.float32, name="denB",
                                        tag="denB", bufs=2)
                        nc.scalar.activation(denA[:], otA[64:65, :], AF.Copy)
                        nc.scalar.activation(denB[:], otB[64:65, :], AF.Copy)
                        rinvA = asb.tile([1, 256], dt.float32, name="rinvA",
                                         tag="rinvA", bufs=2)
                        rinvB = asb.tile([1, 256], dt.float32, name="rinvB",
                                         tag="rinvB", bufs=2)
                        nc.vector.reciprocal_approx_fast(rinvA[:], denA[:])
                        nc.vector.reciprocal_approx_fast(rinvB[:], denB[:])
                        rinvAr = asb.tile([1, 256], dt.float32r, name="rinvAr",
                                          tag="rinvAr", bufs=2)
                        rinvBr = asb.tile([1, 256], dt.float32r, name="rinvBr",
                                          tag="rinvBr", bufs=2)
                        nc.scalar.activation(rinvAr[:], rinvA[:], AF.Copy)
                        nc.scalar.activation(rinvBr[:], rinvB[:], AF.Copy)
                        rb = psA.tile([128, 256], dt.float32, name="rb", tag="rb", bufs=1)
                        nc.tensor.matmul(rb[:], selA[:], rinvAr[:], start=True, stop=False)
                        nc.tensor.matmul(rb[:], selB[:], rinvBr[:], start=False, stop=True)
                        rbs = asb.tile([128, 256], dt.float32, name="rbs", tag="rbs", bufs=2)
                        nc.scalar.activation(rbs[:], rb[:], AF.Copy)
                        dst = at[:, hp * R + b * 256: hp * R + (b + 1) * 256]
                        nc.vector.tensor_mul(dst[0:64, :], otA[0:64, :], rbs[0:64, :])
                        nc.vector.tensor_mul(dst[64:128, :], otB[0:64, :], rbs[64:128, :])

        def proj_residual(wt, bias_name, psp):
            for pe in range(EC):
                pp = psp.tile([128, R], dt.float32, name="pp", tag="qkvp", bufs=2)
                for hp in range(HP):
                    nc.tensor.matmul(pp[:], wt[:, hp * E + pe * 128: hp * E + (pe + 1) * 128],
                                     at[:, hp * R:(hp + 1) * R],
                                     start=(hp == 0), stop=(hp == HP - 1))
                xc = xt[:, pe * R:(pe + 1) * R]
                nc.vector.scalar_tensor_tensor(xc, pp[:], gcol(bias_name, pe), xc,
                                               op0=OP.add, op1=OP.add)

        # ================= program =================
        # LN1 + self KV -> AG(self);  cross KV -> AG(cross);  qT self.
        layer_norm("ln1_g", "ln1_b")
        with tc.tile_pool(name="qkvps", bufs=1, space="PSUM") as psp, \
             tc.tile_pool(name="wpool", bufs=2) as wp, \
             tc.tile_pool(name="kvsb", bufs=1) as kvsb:
            kbuf = kvsb.tile([128, HP * R], dt.bfloat16, name="kbuf", tag="kv", bufs=2)
            vbuf = kvsb.tile([128, 4 * E], dt.bfloat16, name="vbuf", tag="kv", bufs=2)
            wk = load_weight(wp, wts["Wk_s"], "wk_s")
            qkv_transposed(wk, ht, kbuf, psp, kvsb)
            store_kv_k = kbuf
            wv = load_weight(wp, wts["Wv_s"], "wv_s")
            v_normal(wv, ht, vbuf, psp, kvsb)
            store_kv(store_kv_k, vbuf, kv_own_s)
            nc.gpsimd.collective_compute(
                "AllGather", OP.bypass,
                replica_groups=[list(range(NCORES))],
                ins=[kv_own_s[:]], outs=[kv_ag_s[:]])

            wq = load_weight(wp, wts["Wq_s"], "wq_s")
            qkv_transposed(wq, ht, qt, psp, kvsb)

            # cross K/V from context (independent of x) — overlaps self-attn wait
            kbuf2 = kvsb.tile([128, HP * R], dt.bfloat16, name="kbuf2", tag="kv", bufs=2)
            vbuf2 = kvsb.tile([128, 4 * E], dt.bfloat16, name="vbuf2", tag="kv", bufs=2)
            wkc = load_weight(wp, wts["Wk_c"], "wk_c")
            qkv_transposed(wkc, ctx, kbuf2, psp, kvsb)
            wvc = load_weight(wp, wts["Wv_c"], "wv_c")
            v_normal(wvc, ctx, vbuf2, psp, kvsb)
            store_kv(kbuf2, vbuf2, kv_own_c)
            nc.gpsimd.collective_compute(
                "AllGather", OP.bypass,
                replica_groups=[list(range(NCORES))],
                ins=[kv_own_c[:]], outs=[kv_ag_c[:]])

        # self-attention
        attention(kv_ag_s, use_mask=True)
        with tc.tile_pool(name="prjps", bufs=1, space="PSUM") as psp, \
             tc.tile_pool(name="wpool2", bufs=2) as wp:
            wps = load_weight(wp, wts["proj_s_w"], "wproj_s")
            proj_residual(wps, "proj_s_b", psp)

        # cross-attention
        layer_norm("ln2_g", "ln2_b")
        with tc.tile_pool(name="qkvps2", bufs=1, space="PSUM") as psp, \
             tc.tile_pool(name="wpool3", bufs=2) as wp:
            wqc = load_weight(wp, wts["Wq_c"], "wq_c")
            qkv_transposed(wqc, ht, qt, psp, None)
        attention(kv_ag_c, use_mask=False)
        with tc.tile_pool(name="prjps2", bufs=1, space="PSUM") as psp, \
             tc.tile_pool(name="wpool4", bufs=2) as wp:
            wpc = load_weight(wp, wts["proj_c_w"], "wproj_c")
            proj_residual(wpc, "proj_c_b", psp)

        # FFN
        layer_norm("ln3_g", "ln3_b")
        with tc.tile_pool(name="ffnps", bufs=1, space="PSUM") as psp, \
             tc.tile_pool(name="ffnsb", bufs=1) as fsb:
            h4g = fsb.tile([128, FE * R], dt.float32r, name="h4g")
            for fe in range(FE):
                w1t = fsb.tile([128, E], dt.float32r, name="w1t", tag="w1t", bufs=2)
                nc.sync.dma_start(
                    w1t[:],
                    w1[:, fe * 128:(fe + 1) * 128].rearrange("(ec p) c -> p ec c", p=128))
                ph = psp.tile([128, R], dt.float32, name="ph", tag="qkvp", bufs=2)
                for e in range(EC):
                    nc.tensor.matmul(ph[:], w1t[:, e * 128:(e + 1) * 128],
                                     ht[:, e * R:(e + 1) * R],
                                     start=(e == 0), stop=(e == EC - 1))
                nc.scalar.activation(h4g[:, fe * R:(fe + 1) * R], ph[:], AF.Gelu,
                                     bias=b1t[:, fe:fe + 1])
            for pe in range(EC):
                w2t = fsb.tile([128, FE * 128], dt.float32r, name="w2t", tag="w2t", bufs=2)
                nc.sync.dma_start(
                    w2t[:],
                    w2[:, pe * 128:(pe + 1) * 128].rearrange("(fc p) c -> p fc c", p=128))
                po = psp.tile([128, R], dt.float32, name="po", tag="qkvp", bufs=2)
                for fe in range(FE):
                    nc.tensor.matmul(po[:], w2t[:, fe * 128:(fe + 1) * 128],
                                     h4g[:, fe * R:(fe + 1) * R],
                                     start=(fe == 0), stop=(fe == FE - 1))
                xc = xt[:, pe * R:(pe + 1) * R]
                nc.vector.scalar_tensor_tensor(xc, po[:], gcol("b2", pe), xc,
                                               op0=OP.add, op1=OP.add)
                nc.sync.dma_start(outT[pe * 128:(pe + 1) * 128, :], xc)


# ---------------- host side ----------------

_CACHED_NC = None


def _get_nc():
    global _CACHED_NC
    if _CACHED_NC is None:
        _CACHED_NC = _build()
    return _CACHED_NC


def _row_slices(core):
    """Per-batch (lo, hi) T-chunk row ranges owned by `core`."""
    lo, hi = core, 15 - core
    return (slice(lo * 128, (lo + 1) * 128), slice(hi * 128, (hi + 1) * 128))


def _shard_inputs(inputs):
    f32 = np.float32
    x = np.asarray(inputs["x"], f32)
    context = np.asarray(inputs["context"], f32)
    reshape_small = lambda v: np.ascontiguousarray(
        np.asarray(v, f32).reshape(-1, 128).T)
    small_r = {
        "ln1_g_r": reshape_small(inputs["ln1_g"]), "ln1_b_r": reshape_small(inputs["ln1_b"]),
        "ln2_g_r": reshape_small(inputs["ln2_g"]), "ln2_b_r": reshape_small(inputs["ln2_b"]),
        "ln3_g_r": reshape_small(inputs["ln3_g"]), "ln3_b_r": reshape_small(inputs["ln3_b"]),
        "proj_s_b_r": reshape_small(inputs["proj_s_b"]),
        "proj_c_b_r": reshape_small(inputs["proj_c_b"]),
        "b2_r": reshape_small(inputs["b2"]),
        "b1_r": reshape_small(inputs["b1"]),
    }
    selAB = np.zeros((2, 128), f32)
    selAB[0, 0:64] = 1.0
    selAB[1, 64:128] = 1.0
    shared = {
        "ones_col": np.ones((128, 1), f32),
        "ones_row": np.ones((1, 128), f32),
        "selAB_in": selAB,
        "Wq_s": np.asarray(inputs["Wq_s"], f32), "Wk_s": np.asarray(inputs["Wk_s"], f32),
        "Wv_s": np.asarray(inputs["Wv_s"], f32), "proj_s_w": np.asarray(inputs["proj_s_w"], f32),
        "Wq_c": np.asarray(inputs["Wq_c"], f32), "Wk_c": np.asarray(inputs["Wk_c"], f32),
        "Wv_c": np.asarray(inputs["Wv_c"], f32), "proj_c_w": np.asarray(inputs["proj_c_w"], f32),
        "w1": np.asarray(inputs["w1"], f32), "w2": np.asarray(inputs["w2"], f32),
        **small_r,
    }
    in_maps = []
    kk = np.arange(T)[:, None]
    for core in range(NCORES):
        lo_sl, hi_sl = _row_slices(core)
        xrows = np.concatenate(
            [x[0, lo_sl], x[0, hi_sl], x[1, lo_sl], x[1, hi_sl]], axis=0)
        ctxrows = np.concatenate(
            [context[0, core * 256:(core + 1) * 256],
             context[1, core * 256:(core + 1) * 256]], axis=0)
        qpos = np.concatenate([np.arange(core * 128, (core + 1) * 128),
                               np.arange((15 - core) * 128, (16 - core) * 128)])
        maskbin = (kk <= qpos[None, :]).astype(f32)  # [T, 256] in natural key order
        mchunks = maskbin.reshape(CH, 128, 256)
        pi = [j if j < 8 else 15 - (j - 8) for j in range(CH)]
        mchunks = mchunks[pi]  # storage-block order (rank-major)
        # mbh[p, j*256+q] = mask of block j
        mbh = np.ascontiguousarray(
            mchunks.transpose(1, 0, 2).reshape(128, CH * 256)).astype(BF)
        in_maps.append({
            "xT": np.ascontiguousarray(xrows.T),
            "ctxT": np.ascontiguousarray(ctxrows.T),
            "mbh": mbh,
            **shared,
        })
    return in_maps


def _unshard_output(results):
    out = np.empty((B, T, E), np.float32)
    for core in range(NCORES):
        rows = results[core]["outT"].T  # [512, E]
        lo_sl, hi_sl = _row_slices(core)
        out[0, lo_sl] = rows[0:128]
        out[0, hi_sl] = rows[128:256]
        out[1, lo_sl] = rows[256:384]
        out[1, hi_sl] = rows[384:512]
    return out


def kernel(**inputs):
    nc = _get_nc()
    in_maps = _shard_inputs(inputs)
    res = bass_utils.run_bass_kernel_spmd(nc, in_maps, core_ids=list(range(NCORES)))
    return _unshard_output(res.results)


if __name__ == "__main__":
    # smoke test with random inputs
    rng = np.random.default_rng(0)
    dummy = {
        "x": rng.standard_normal((B, T, E), dtype=np.float32),
        "context": rng.standard_normal((B, T, E), dtype=np.float32),
    }
    for n in ["ln1", "ln2", "ln3"]:
        dummy[n + "_g"] = np.ones(E, np.float32)
        dummy[n + "_b"] = np.zeros(E, np.float32)
    for n in ["Wq_s", "Wk_s", "Wv_s", "proj_s_w", "Wq_c", "Wk_c", "Wv_c", "proj_c_w"]:
        dummy[n] = (rng.standard_normal((E, E), dtype=np.float32) * 0.02)
    dummy["proj_s_b"] = np.zeros(E, np.float32)
    dummy["proj_c_b"] = np.zeros(E, np.float32)
    dummy["w1"] = rng.standard_normal((E, 4 * E), dtype=np.float32) * 0.02
    dummy["b1"] = np.zeros(4 * E, np.float32)
    dummy["w2"] = rng.standard_normal((4 * E, E), dtype=np.float32) * 0.02
    dummy["b2"] = np.zeros(E, np.float32)
    out = kernel(**dummy)
    print("out", out.shape, out.dtype, np.abs(out).mean())
